# revision 18
# baseline (speedup 1.0000x reference)
"""GumbelVectorQuantizer (eval/hard-argmax) forward on 8 Trainium2 cores.

Data-parallel over batch: each core handles 2 of 16 batches (6000 tokens).
Per core:
  logits = x @ W.T (+b)            PE, K-chunked fp32 PSUM accumulation
  argmax per group                 DVE reduce_max + is_equal onehot . iota
  softmax-sum (avg_probs partial)  ACT exp(accum sumexp) + PE matmul(rse, exps)
  quantized = codebook[argmax]     SWDGE indirect DMA gather
avg_probs partials are summed on host (equivalent to the all-reduce).
"""

import sys

import numpy as np

if "/opt/trn_rl_repo" not in sys.path:
    sys.path.insert(0, "/opt/trn_rl_repo")

from contextlib import ExitStack

import concourse.bass as bass
import concourse.tile as tile
from concourse import bacc, mybir
from concourse.bass_utils import run_bass_kernel_spmd

B, T, D = 16, 3000, 512
GROUPS, NUM_VARS = 2, 320
VQ_DIM = 256
VAR_DIM = VQ_DIM // GROUPS  # 128
GV = GROUPS * NUM_VARS  # 640

N_CORES = 8
NTOK = (B // N_CORES) * T  # 6000 tokens per core
KCH = D // 128  # 4 contraction chunks
TT = 512  # tokens per outer tile
N_OUTER = (NTOK + TT - 1) // TT  # 12 (last tile ragged: 368)

F32 = mybir.dt.float32
I32 = mybir.dt.int32
AF = mybir.ActivationFunctionType
OP = mybir.AluOpType


def _build_program(
    has_bias: bool,
    do_gather: bool = True,
    do_argmax: bool = True,
    do_avg: bool = True,
    do_softmax: bool = True,
) -> bass.Bass:
    nc = bacc.Bacc("TRN2", target_bir_lowering=False, debug=False)

    xT = nc.dram_tensor("xT", [D, NTOK], F32, kind="ExternalInput").ap()
    WT = nc.dram_tensor("WT", [D, GV], F32, kind="ExternalInput").ap()
    cb = nc.dram_tensor("cb", [GV, VAR_DIM], F32, kind="ExternalInput").ap()
    bv = (
        nc.dram_tensor("bv", [1, GV], F32, kind="ExternalInput").ap()
        if has_bias
        else None
    )
    q_out = nc.dram_tensor("q_out", [NTOK, VQ_DIM], F32, kind="ExternalOutput").ap()
    avgp = nc.dram_tensor("avgp", [GROUPS, NUM_VARS], F32, kind="ExternalOutput").ap()

    with tile.TileContext(nc) as tc, ExitStack() as ctx:
        const_p = ctx.enter_context(tc.tile_pool(name="const", bufs=1))
        x_p = ctx.enter_context(tc.tile_pool(name="x", bufs=2))
        lg_p = ctx.enter_context(tc.tile_pool(name="lg", bufs=2, space="PSUM"))
        avg_p = ctx.enter_context(tc.tile_pool(name="avg", bufs=1, space="PSUM"))
        sm_p = ctx.enter_context(tc.tile_pool(name="sm", bufs=2))
        oh_p = ctx.enter_context(tc.tile_pool(name="oh", bufs=2))
        small_p = ctx.enter_context(tc.tile_pool(name="small", bufs=4))
        q_p = ctx.enter_context(tc.tile_pool(name="q", bufs=2))
        idx_p = ctx.enter_context(tc.tile_pool(name="idx", bufs=2))

        # Constants resident for the whole kernel.
        wt_t = const_p.tile([128, KCH, GV], F32, tag="wt")
        nc.sync.dma_start(out=wt_t[:], in_=WT.rearrange("(k p) v -> p k v", p=128))
        # Per-group iota whose base folds in the group's row offset in the
        # combined [GV, VAR_DIM] codebook, so onehot . iota is directly the
        # gather row index.
        iota_ts = []
        for g in range(GROUPS):
            it = const_p.tile([128, NUM_VARS], F32, tag=f"iota{g}")
            nc.gpsimd.iota(
                it[:],
                pattern=[[1, NUM_VARS]],
                base=g * NUM_VARS,
                channel_multiplier=0,
                allow_small_or_imprecise_dtypes=True,
            )
            iota_ts.append(it)
        if has_bias:
            b_t = const_p.tile([1, GV], F32, tag="bv")
            nc.sync.dma_start(out=b_t[:], in_=bv[:, :])
            ones_t = const_p.tile([1, 128], F32, tag="ones")
            nc.gpsimd.memset(ones_t[:], 1.0)

        # avg_ps[g] accumulates sum over tokens of softmax probs; only row g
        # is meaningful (lhsT has both groups' 1/sumexp columns).
        avg_ps = [
            avg_p.tile([GROUPS, NUM_VARS], F32, tag=f"avg{g}", name=f"avg{g}")
            for g in range(GROUPS)
        ]

        for t in range(N_OUTER):
            tok0 = t * TT
            tsz = min(TT, NTOK - tok0)
            nsub = (tsz + 127) // 128

            x_t = x_p.tile([128, KCH, TT], F32, tag="x")
            nc.sync.dma_start(
                out=x_t[:, :, :tsz],
                in_=xT[:, tok0 : tok0 + tsz].rearrange("(k p) t -> p k t", p=128),
            )

            idxf = idx_p.tile([128, 2 * 4], F32, tag="idxf")
            idxi = idx_p.tile([128, 2 * 4], I32, tag="idxi")
            if tsz % 128 != 0:
                # ragged final subtile: rows past ssz would hold garbage
                # indices; zero them so the gather stays in bounds.
                nc.gpsimd.memset(idxf[:], 0.0)
            q_t = q_p.tile([128, 2 * 4, VAR_DIM], F32, tag="q")

            for s in range(nsub):
                s0 = s * 128
                ssz = min(128, tsz - s0)
                last = t == N_OUTER - 1 and s == nsub - 1

                lg = [
                    lg_p.tile([128, NUM_VARS], F32, tag=f"lg{g}", name=f"lg{g}")
                    for g in range(GROUPS)
                ]
                for g in range(GROUPS):
                    vs = slice(g * NUM_VARS, (g + 1) * NUM_VARS)
                    if has_bias:
                        nc.tensor.matmul(
                            out=lg[g][:ssz, :],
                            lhsT=ones_t[0:1, :ssz],
                            rhs=b_t[0:1, vs],
                            start=True,
                            stop=False,
                        )
                    for k in range(KCH):
                        nc.tensor.matmul(
                            out=lg[g][:ssz, :],
                            lhsT=x_t[:, k, s0 : s0 + ssz],
                            rhs=wt_t[:, k, vs],
                            start=(k == 0 and not has_bias),
                            stop=(k == KCH - 1),
                        )

                mx = small_p.tile([128, GROUPS], F32, tag="mx")
                negmx = small_p.tile([128, GROUPS], F32, tag="negmx")
                se = small_p.tile([128, GROUPS], F32, tag="se")
                rse = small_p.tile([128, GROUPS], F32, tag="rse")
                exps = sm_p.tile([128, GV], F32, tag="exps")

                for g in range(GROUPS):
                    nc.vector.tensor_reduce(
                        out=mx[:ssz, g : g + 1],
                        in_=lg[g][:ssz, :],
                        axis=mybir.AxisListType.X,
                        op=OP.max,
                    )
                if do_softmax:
                    nc.vector.tensor_scalar_mul(negmx[:ssz, :], mx[:ssz, :], -1.0)
                for g in range(GROUPS):
                    if do_softmax:
                        nc.scalar.activation(
                            out=exps[:ssz, g * NUM_VARS : (g + 1) * NUM_VARS],
                            in_=lg[g][:ssz, :],
                            func=AF.Exp,
                            bias=negmx[:ssz, g : g + 1],
                            scale=1.0,
                            accum_out=se[:ssz, g : g + 1],
                        )
                    if do_argmax:
                        # idx = sum_v (lg_v == mx) * iota_v in a single DVE op;
                        # iota's base already folds in the group's codebook row
                        # offset.
                        scr = oh_p.tile([128, NUM_VARS], F32, tag="scr")
                        nc.vector.scalar_tensor_tensor(
                            out=scr[:ssz, :],
                            in0=lg[g][:ssz, :],
                            scalar=mx[:ssz, g : g + 1],
                            in1=iota_ts[g][:ssz, :],
                            op0=OP.is_equal,
                            op1=OP.mult,
                            accum_out=idxf[:ssz, s * GROUPS + g : s * GROUPS + g + 1],
                        )

                if do_softmax:
                    nc.vector.reciprocal(rse[:ssz, :], se[:ssz, :])
                if do_avg and do_softmax:
                    for g in range(GROUPS):
                        nc.tensor.matmul(
                            out=avg_ps[g][:GROUPS, :],
                            lhsT=rse[:ssz, :],
                            rhs=exps[:ssz, g * NUM_VARS : (g + 1) * NUM_VARS],
                            start=(t == 0 and s == 0),
                            stop=last,
                        )

            if do_argmax:
                nc.vector.tensor_copy(out=idxi[:, : 2 * nsub], in_=idxf[:, : 2 * nsub])
            else:
                nc.gpsimd.memset(idxi[:], 0)

            if do_gather:
                # multi-index-per-partition offsets crash the device; one
                # gather per (subtile, group) column.
                for j in range(2 * nsub):
                    nc.gpsimd.indirect_dma_start(
                        out=q_t[:, j, :],
                        out_offset=None,
                        in_=cb[:, :],
                        in_offset=bass.IndirectOffsetOnAxis(
                            ap=idxi[:, j : j + 1], axis=0
                        ),
                    )
            else:
                nc.gpsimd.memset(q_t[:], 0.0)

            for s in range(nsub):
                s0 = s * 128
                ssz = min(128, tsz - s0)
                nc.sync.dma_start(
                    out=q_out[tok0 + s0 : tok0 + s0 + ssz, :].rearrange(
                        "t (g d) -> t g d", d=VAR_DIM
                    ),
                    in_=q_t[:ssz, s * GROUPS : (s + 1) * GROUPS, :],
                )

        # DVE reads must start at partition 0, so copy both PSUM rows per
        # group and let the output DMA select row g.
        avg_sb = const_p.tile([GROUPS, GROUPS * NUM_VARS], F32, tag="avg_sb")
        if not (do_avg and do_softmax):
            nc.gpsimd.memset(avg_sb[:], 0.0)
        for g in range(GROUPS):
            if do_avg and do_softmax:
                nc.vector.tensor_copy(
                    out=avg_sb[:GROUPS, g * NUM_VARS : (g + 1) * NUM_VARS],
                    in_=avg_ps[g][:GROUPS, :],
                )
            nc.sync.dma_start(
                out=avgp[g : g + 1, :],
                in_=avg_sb[g : g + 1, g * NUM_VARS : (g + 1) * NUM_VARS],
            )

    nc.compile()
    return nc


_PROGRAM_CACHE: dict[bool, bass.Bass] = {}


def _prepare(x, W, b, codebook):
    x = np.ascontiguousarray(np.asarray(x, dtype=np.float32))
    W = np.asarray(W, dtype=np.float32)
    b = np.asarray(b, dtype=np.float32)
    cbk = np.asarray(codebook, dtype=np.float32).reshape(GV, VAR_DIM)
    has_bias = bool(np.any(b != 0.0))

    WTh = np.ascontiguousarray(W.T)  # [D, GV]
    cbc = np.ascontiguousarray(cbk)  # [GV, VAR_DIM]
    per_b = B // N_CORES
    in_maps = []
    for i in range(N_CORES):
        xs = x[i * per_b : (i + 1) * per_b].reshape(NTOK, D)
        m = {
            "xT": np.ascontiguousarray(xs.T),
            "WT": WTh,
            "cb": cbc,
        }
        if has_bias:
            m["bv"] = np.ascontiguousarray(b.reshape(1, GV))
        in_maps.append(m)
    return has_bias, in_maps


def _postprocess(results):
    q = np.concatenate([results[i]["q_out"] for i in range(N_CORES)], axis=0)
    q = q.reshape(B, T, VQ_DIM)
    avg = np.zeros((GROUPS, NUM_VARS), dtype=np.float64)
    for i in range(N_CORES):
        avg += results[i]["avgp"].astype(np.float64)
    avg_probs = (avg / (B * T)).astype(np.float32)
    return q, avg_probs


def _get_program(has_bias: bool) -> bass.Bass:
    nc = _PROGRAM_CACHE.get(has_bias)
    if nc is None:
        nc = _build_program(has_bias)
        _PROGRAM_CACHE[has_bias] = nc
    return nc


def kernel(x, W, b, codebook):
    has_bias, in_maps = _prepare(x, W, b, codebook)
    nc = _get_program(has_bias)
    results = run_bass_kernel_spmd(nc, in_maps, list(range(N_CORES))).results
    return _postprocess(results)


# revision 25
# speedup vs baseline: 1.1711x; 1.1711x over previous
"""GumbelVectorQuantizer (eval/hard-argmax) forward on 8 Trainium2 cores.

Data-parallel over batch: each core handles 2 of 16 batches (6000 tokens).
Per core:
  logits = x @ W.T (+b)            PE, K-chunked fp32 PSUM accumulation
  argmax per group                 DVE reduce_max + is_equal onehot . iota
  softmax-sum (avg_probs partial)  ACT exp(accum sumexp) + PE matmul(rse, exps)
  quantized = codebook[argmax]     SWDGE indirect DMA gather
avg_probs partials are summed on host (equivalent to the all-reduce).
"""

import sys

import numpy as np

if "/opt/trn_rl_repo" not in sys.path:
    sys.path.insert(0, "/opt/trn_rl_repo")

from contextlib import ExitStack

import concourse.bass as bass
import concourse.tile as tile
from concourse import bacc, mybir
from concourse.bass_utils import run_bass_kernel_spmd

B, T, D = 16, 3000, 512
GROUPS, NUM_VARS = 2, 320
VQ_DIM = 256
VAR_DIM = VQ_DIM // GROUPS  # 128
GV = GROUPS * NUM_VARS  # 640

N_CORES = 8
NTOK = (B // N_CORES) * T  # 6000 tokens per core
KCH = D // 128  # 4 contraction chunks
TT = 512  # tokens per outer tile
N_OUTER = (NTOK + TT - 1) // TT  # 12 (last tile ragged: 368)

F32 = mybir.dt.float32
BF16 = mybir.dt.bfloat16
I32 = mybir.dt.int32
AF = mybir.ActivationFunctionType
OP = mybir.AluOpType


def _build_program(
    has_bias: bool,
    do_gather: bool = True,
    do_argmax: bool = True,
    do_avg: bool = True,
    do_softmax: bool = True,
) -> bass.Bass:
    nc = bacc.Bacc("TRN2", target_bir_lowering=False, debug=False)

    xT = nc.dram_tensor("xT", [D, NTOK], F32, kind="ExternalInput").ap()
    WT = nc.dram_tensor("WT", [D, GV], F32, kind="ExternalInput").ap()
    cb = nc.dram_tensor("cb", [GV, VAR_DIM], F32, kind="ExternalInput").ap()
    bv = (
        nc.dram_tensor("bv", [1, GV], F32, kind="ExternalInput").ap()
        if has_bias
        else None
    )
    q_out = nc.dram_tensor("q_out", [NTOK, VQ_DIM], F32, kind="ExternalOutput").ap()
    avgp = nc.dram_tensor("avgp", [GROUPS, NUM_VARS], F32, kind="ExternalOutput").ap()

    with tile.TileContext(nc) as tc, ExitStack() as ctx:
        const_p = ctx.enter_context(tc.tile_pool(name="const", bufs=1))
        x_p = ctx.enter_context(tc.tile_pool(name="x", bufs=2))
        lg_p = ctx.enter_context(tc.tile_pool(name="lg", bufs=3, space="PSUM"))
        avg_p = ctx.enter_context(tc.tile_pool(name="avg", bufs=1, space="PSUM"))
        # exps/rse live until the end-of-tile avg matmuls: nsub alive + margin
        sm_p = ctx.enter_context(tc.tile_pool(name="sm", bufs=6))
        oh_p = ctx.enter_context(tc.tile_pool(name="oh", bufs=2))
        small_p = ctx.enter_context(tc.tile_pool(name="small", bufs=4))
        rse_p = ctx.enter_context(tc.tile_pool(name="rse", bufs=6))
        q_p = ctx.enter_context(tc.tile_pool(name="q", bufs=2))
        idx_p = ctx.enter_context(tc.tile_pool(name="idx", bufs=2))

        # Constants resident for the whole kernel.
        wt_t = const_p.tile([128, KCH, GV], F32, tag="wt")
        nc.sync.dma_start(out=wt_t[:], in_=WT.rearrange("(k p) v -> p k v", p=128))
        # Per-group iota whose base folds in the group's row offset in the
        # combined [GV, VAR_DIM] codebook, so onehot . iota is directly the
        # gather row index.
        iota_ts = []
        for g in range(GROUPS):
            it = const_p.tile([128, NUM_VARS], F32, tag=f"iota{g}")
            nc.gpsimd.iota(
                it[:],
                pattern=[[1, NUM_VARS]],
                base=g * NUM_VARS,
                channel_multiplier=0,
                allow_small_or_imprecise_dtypes=True,
            )
            iota_ts.append(it)
        if has_bias:
            b_t = const_p.tile([1, GV], F32, tag="bv")
            nc.sync.dma_start(out=b_t[:], in_=bv[:, :])
            ones_t = const_p.tile([1, 128], F32, tag="ones")
            nc.gpsimd.memset(ones_t[:], 1.0)

        # avg_ps[g] accumulates sum over tokens of softmax probs; only row g
        # is meaningful (lhsT has both groups' 1/sumexp columns).
        avg_ps = [
            avg_p.tile([GROUPS, NUM_VARS], F32, tag=f"avg{g}", name=f"avg{g}")
            for g in range(GROUPS)
        ]

        for t in range(N_OUTER):
            tok0 = t * TT
            tsz = min(TT, NTOK - tok0)
            nsub = (tsz + 127) // 128

            x_t = x_p.tile([128, KCH, TT], F32, tag="x")
            nc.sync.dma_start(
                out=x_t[:, :, :tsz],
                in_=xT[:, tok0 : tok0 + tsz].rearrange("(k p) t -> p k t", p=128),
            )

            idxf = idx_p.tile([128, 2 * 4], F32, tag="idxf")
            idxi = idx_p.tile([128, 2 * 4], I32, tag="idxi")
            if tsz % 128 != 0:
                # ragged final subtile: rows past ssz would hold garbage
                # indices; zero them so the gather stays in bounds.
                nc.gpsimd.memset(idxf[:], 0.0)
            q_t = q_p.tile([128, 2 * 4, VAR_DIM], F32, tag="q")

            exps_list = []
            rse_list = []
            for s in range(nsub):
                s0 = s * 128
                ssz = min(128, tsz - s0)

                lg = [
                    lg_p.tile([128, NUM_VARS], F32, tag=f"lg{g}", name=f"lg{g}")
                    for g in range(GROUPS)
                ]
                for g in range(GROUPS):
                    vs = slice(g * NUM_VARS, (g + 1) * NUM_VARS)
                    if has_bias:
                        nc.tensor.matmul(
                            out=lg[g][:ssz, :],
                            lhsT=ones_t[0:1, :ssz],
                            rhs=b_t[0:1, vs],
                            start=True,
                            stop=False,
                        )
                    for k in range(KCH):
                        nc.tensor.matmul(
                            out=lg[g][:ssz, :],
                            lhsT=x_t[:, k, s0 : s0 + ssz],
                            rhs=wt_t[:, k, vs],
                            start=(k == 0 and not has_bias),
                            stop=(k == KCH - 1),
                        )

                mx = small_p.tile([128, GROUPS], F32, tag="mx")
                negmx = small_p.tile([128, GROUPS], F32, tag="negmx")
                se = small_p.tile([128, GROUPS], F32, tag="se")
                rse = rse_p.tile([128, GROUPS], BF16, tag="rse")
                exps = sm_p.tile([128, GV], BF16, tag="exps")
                exps_list.append(exps)
                rse_list.append(rse)

                for g in range(GROUPS):
                    nc.vector.tensor_reduce(
                        out=mx[:ssz, g : g + 1],
                        in_=lg[g][:ssz, :],
                        axis=mybir.AxisListType.X,
                        op=OP.max,
                    )
                if do_softmax:
                    nc.vector.tensor_scalar_mul(negmx[:ssz, :], mx[:ssz, :], -1.0)
                for g in range(GROUPS):
                    if do_softmax:
                        nc.scalar.activation(
                            out=exps[:ssz, g * NUM_VARS : (g + 1) * NUM_VARS],
                            in_=lg[g][:ssz, :],
                            func=AF.Exp,
                            bias=negmx[:ssz, g : g + 1],
                            scale=1.0,
                            accum_out=se[:ssz, g : g + 1],
                        )
                    if do_argmax:
                        # idx = sum_v (lg_v == mx) * iota_v in a single DVE op;
                        # iota's base already folds in the group's codebook row
                        # offset.
                        scr = oh_p.tile([128, NUM_VARS], F32, tag="scr")
                        nc.vector.scalar_tensor_tensor(
                            out=scr[:ssz, :],
                            in0=lg[g][:ssz, :],
                            scalar=mx[:ssz, g : g + 1],
                            in1=iota_ts[g][:ssz, :],
                            op0=OP.is_equal,
                            op1=OP.mult,
                            accum_out=idxf[:ssz, s * GROUPS + g : s * GROUPS + g + 1],
                        )

                if do_softmax:
                    with nc.allow_low_precision(
                        reason="1/sumexp in bf16 feeds an averaged matmul"
                    ):
                        nc.vector.reciprocal(rse[:ssz, :], se[:ssz, :])

            if do_avg and do_softmax:
                # Batched after the subtile loop so the PE never stalls
                # waiting on the softmax chain mid-stream; inputs are bf16
                # (plenty for an average of probabilities) so these matmuls
                # run at full rate.
                for s in range(nsub):
                    ssz = min(128, tsz - s * 128)
                    last = t == N_OUTER - 1 and s == nsub - 1
                    for g in range(GROUPS):
                        nc.tensor.matmul(
                            out=avg_ps[g][:GROUPS, :],
                            lhsT=rse_list[s][:ssz, :],
                            rhs=exps_list[s][:ssz, g * NUM_VARS : (g + 1) * NUM_VARS],
                            start=(t == 0 and s == 0),
                            stop=last,
                        )

            if do_argmax:
                nc.vector.tensor_copy(out=idxi[:, : 2 * nsub], in_=idxf[:, : 2 * nsub])
            else:
                nc.gpsimd.memset(idxi[:], 0)

            if do_gather:
                # multi-index-per-partition offsets crash the device; one
                # gather per (subtile, group) column.
                for j in range(2 * nsub):
                    nc.gpsimd.indirect_dma_start(
                        out=q_t[:, j, :],
                        out_offset=None,
                        in_=cb[:, :],
                        in_offset=bass.IndirectOffsetOnAxis(
                            ap=idxi[:, j : j + 1], axis=0
                        ),
                    )
            else:
                nc.gpsimd.memset(q_t[:], 0.0)

            for s in range(nsub):
                s0 = s * 128
                ssz = min(128, tsz - s0)
                nc.sync.dma_start(
                    out=q_out[tok0 + s0 : tok0 + s0 + ssz, :].rearrange(
                        "t (g d) -> t g d", d=VAR_DIM
                    ),
                    in_=q_t[:ssz, s * GROUPS : (s + 1) * GROUPS, :],
                )

        # DVE reads must start at partition 0, so copy both PSUM rows per
        # group and let the output DMA select row g.
        avg_sb = const_p.tile([GROUPS, GROUPS * NUM_VARS], F32, tag="avg_sb")
        if not (do_avg and do_softmax):
            nc.gpsimd.memset(avg_sb[:], 0.0)
        for g in range(GROUPS):
            if do_avg and do_softmax:
                nc.vector.tensor_copy(
                    out=avg_sb[:GROUPS, g * NUM_VARS : (g + 1) * NUM_VARS],
                    in_=avg_ps[g][:GROUPS, :],
                )
            nc.sync.dma_start(
                out=avgp[g : g + 1, :],
                in_=avg_sb[g : g + 1, g * NUM_VARS : (g + 1) * NUM_VARS],
            )

    nc.compile()
    return nc


_PROGRAM_CACHE: dict[bool, bass.Bass] = {}


def _prepare(x, W, b, codebook):
    x = np.ascontiguousarray(np.asarray(x, dtype=np.float32))
    W = np.asarray(W, dtype=np.float32)
    b = np.asarray(b, dtype=np.float32)
    cbk = np.asarray(codebook, dtype=np.float32).reshape(GV, VAR_DIM)
    has_bias = bool(np.any(b != 0.0))

    WTh = np.ascontiguousarray(W.T)  # [D, GV]
    cbc = np.ascontiguousarray(cbk)  # [GV, VAR_DIM]
    per_b = B // N_CORES
    in_maps = []
    for i in range(N_CORES):
        xs = x[i * per_b : (i + 1) * per_b].reshape(NTOK, D)
        m = {
            "xT": np.ascontiguousarray(xs.T),
            "WT": WTh,
            "cb": cbc,
        }
        if has_bias:
            m["bv"] = np.ascontiguousarray(b.reshape(1, GV))
        in_maps.append(m)
    return has_bias, in_maps


def _postprocess(results):
    q = np.concatenate([results[i]["q_out"] for i in range(N_CORES)], axis=0)
    q = q.reshape(B, T, VQ_DIM)
    avg = np.zeros((GROUPS, NUM_VARS), dtype=np.float64)
    for i in range(N_CORES):
        avg += results[i]["avgp"].astype(np.float64)
    avg_probs = (avg / (B * T)).astype(np.float32)
    return q, avg_probs


def _get_program(has_bias: bool) -> bass.Bass:
    nc = _PROGRAM_CACHE.get(has_bias)
    if nc is None:
        nc = _build_program(has_bias)
        _PROGRAM_CACHE[has_bias] = nc
    return nc


def kernel(x, W, b, codebook):
    has_bias, in_maps = _prepare(x, W, b, codebook)
    nc = _get_program(has_bias)
    results = run_bass_kernel_spmd(nc, in_maps, list(range(N_CORES))).results
    return _postprocess(results)


# revision 32
# speedup vs baseline: 1.4698x; 1.2551x over previous
"""GumbelVectorQuantizer (eval/hard-argmax) forward on 8 Trainium2 cores.

Data-parallel over batch: each core handles 2 of 16 batches (6000 tokens).
Per core:
  logits = x @ W.T (+b)            PE, K-chunked fp32 PSUM accumulation
  argmax per group                 DVE reduce_max + is_equal onehot . iota
  softmax-sum (avg_probs partial)  ACT exp(accum sumexp) + PE matmul(rse, exps)
  quantized = codebook[argmax]     SWDGE indirect DMA gather
avg_probs partials are summed on host (equivalent to the all-reduce).
"""

import sys

import numpy as np

if "/opt/trn_rl_repo" not in sys.path:
    sys.path.insert(0, "/opt/trn_rl_repo")

from contextlib import ExitStack

import concourse.bass as bass
import concourse.tile as tile
from concourse import bacc, mybir
from concourse.bass_utils import run_bass_kernel_spmd

B, T, D = 16, 3000, 512
GROUPS, NUM_VARS = 2, 320
VQ_DIM = 256
VAR_DIM = VQ_DIM // GROUPS  # 128
GV = GROUPS * NUM_VARS  # 640

N_CORES = 8
NTOK = (B // N_CORES) * T  # 6000 tokens per core
KCH = D // 128  # 4 contraction chunks
TT = 512  # tokens per outer tile
N_OUTER = (NTOK + TT - 1) // TT  # 12 (last tile ragged: 368)

F32 = mybir.dt.float32
BF16 = mybir.dt.bfloat16
I32 = mybir.dt.int32
AF = mybir.ActivationFunctionType
OP = mybir.AluOpType


def _build_program(
    has_bias: bool,
    do_gather: bool = True,
    do_argmax: bool = True,
    do_avg: bool = True,
    do_softmax: bool = True,
    bf16_split: bool = True,
) -> bass.Bass:
    nc = bacc.Bacc("TRN2", target_bir_lowering=False, debug=False)

    if bf16_split:
        xTh = nc.dram_tensor("xTh", [D, NTOK], BF16, kind="ExternalInput").ap()
        xTl = nc.dram_tensor("xTl", [D, NTOK], BF16, kind="ExternalInput").ap()
        WTh = nc.dram_tensor("WTh", [D, GV], BF16, kind="ExternalInput").ap()
        WTl = nc.dram_tensor("WTl", [D, GV], BF16, kind="ExternalInput").ap()
    else:
        xT = nc.dram_tensor("xT", [D, NTOK], F32, kind="ExternalInput").ap()
        WT = nc.dram_tensor("WT", [D, GV], F32, kind="ExternalInput").ap()
    cb = nc.dram_tensor("cb", [GV, VAR_DIM], F32, kind="ExternalInput").ap()
    bv = (
        nc.dram_tensor("bv", [1, GV], F32, kind="ExternalInput").ap()
        if has_bias
        else None
    )
    q_out = nc.dram_tensor("q_out", [NTOK, VQ_DIM], F32, kind="ExternalOutput").ap()
    avgp = nc.dram_tensor("avgp", [GROUPS, NUM_VARS], F32, kind="ExternalOutput").ap()

    with tile.TileContext(nc) as tc, ExitStack() as ctx:
        const_p = ctx.enter_context(tc.tile_pool(name="const", bufs=1))
        x_p = ctx.enter_context(tc.tile_pool(name="x", bufs=2))
        lg_p = ctx.enter_context(tc.tile_pool(name="lg", bufs=3, space="PSUM"))
        avg_p = ctx.enter_context(tc.tile_pool(name="avg", bufs=1, space="PSUM"))
        # exps/rse live until the end-of-tile avg matmuls: nsub alive + margin
        sm_p = ctx.enter_context(tc.tile_pool(name="sm", bufs=6))
        oh_p = ctx.enter_context(tc.tile_pool(name="oh", bufs=2))
        small_p = ctx.enter_context(tc.tile_pool(name="small", bufs=4))
        rse_p = ctx.enter_context(tc.tile_pool(name="rse", bufs=6))
        q_p = ctx.enter_context(tc.tile_pool(name="q", bufs=2))
        idx_p = ctx.enter_context(tc.tile_pool(name="idx", bufs=2))

        # Constants resident for the whole kernel.
        if bf16_split:
            wh_t = const_p.tile([128, KCH, GV], BF16, tag="wh")
            nc.sync.dma_start(out=wh_t[:], in_=WTh.rearrange("(k p) v -> p k v", p=128))
            wl_t = const_p.tile([128, KCH, GV], BF16, tag="wl")
            nc.sync.dma_start(out=wl_t[:], in_=WTl.rearrange("(k p) v -> p k v", p=128))
        else:
            wt_t = const_p.tile([128, KCH, GV], F32, tag="wt")
            nc.sync.dma_start(out=wt_t[:], in_=WT.rearrange("(k p) v -> p k v", p=128))
        # Per-group iota whose base folds in the group's row offset in the
        # combined [GV, VAR_DIM] codebook, so onehot . iota is directly the
        # gather row index.
        iota_ts = []
        for g in range(GROUPS):
            it = const_p.tile([128, NUM_VARS], F32, tag=f"iota{g}")
            nc.gpsimd.iota(
                it[:],
                pattern=[[1, NUM_VARS]],
                base=g * NUM_VARS,
                channel_multiplier=0,
                allow_small_or_imprecise_dtypes=True,
            )
            iota_ts.append(it)
        if has_bias:
            b_t = const_p.tile([1, GV], F32, tag="bv")
            nc.sync.dma_start(out=b_t[:], in_=bv[:, :])
            ones_t = const_p.tile([1, 128], F32, tag="ones")
            nc.gpsimd.memset(ones_t[:], 1.0)

        # avg_ps[g] accumulates sum over tokens of softmax probs; only row g
        # is meaningful (lhsT has both groups' 1/sumexp columns).
        avg_ps = [
            avg_p.tile([GROUPS, NUM_VARS], F32, tag=f"avg{g}", name=f"avg{g}")
            for g in range(GROUPS)
        ]

        for t in range(N_OUTER):
            tok0 = t * TT
            tsz = min(TT, NTOK - tok0)
            nsub = (tsz + 127) // 128

            if bf16_split:
                xh_t = x_p.tile([128, KCH, TT], BF16, tag="xh")
                nc.sync.dma_start(
                    out=xh_t[:, :, :tsz],
                    in_=xTh[:, tok0 : tok0 + tsz].rearrange("(k p) t -> p k t", p=128),
                )
                xl_t = x_p.tile([128, KCH, TT], BF16, tag="xl")
                nc.sync.dma_start(
                    out=xl_t[:, :, :tsz],
                    in_=xTl[:, tok0 : tok0 + tsz].rearrange("(k p) t -> p k t", p=128),
                )
            else:
                x_t = x_p.tile([128, KCH, TT], F32, tag="x")
                nc.sync.dma_start(
                    out=x_t[:, :, :tsz],
                    in_=xT[:, tok0 : tok0 + tsz].rearrange("(k p) t -> p k t", p=128),
                )

            idxf = idx_p.tile([128, 2 * 4], F32, tag="idxf")
            idxi = idx_p.tile([128, 2 * 4], I32, tag="idxi")
            if tsz % 128 != 0:
                # ragged final subtile: rows past ssz would hold garbage
                # indices; zero them so the gather stays in bounds.
                nc.gpsimd.memset(idxf[:], 0.0)
            q_t = q_p.tile([128, 2 * 4, VAR_DIM], F32, tag="q")

            exps_list = []
            rse_list = []
            for s in range(nsub):
                s0 = s * 128
                ssz = min(128, tsz - s0)

                lg = [
                    lg_p.tile([128, NUM_VARS], F32, tag=f"lg{g}", name=f"lg{g}")
                    for g in range(GROUPS)
                ]
                if has_bias:
                    for g in range(GROUPS):
                        nc.tensor.matmul(
                            out=lg[g][:ssz, :],
                            lhsT=ones_t[0:1, :ssz],
                            rhs=b_t[0:1, g * NUM_VARS : (g + 1) * NUM_VARS],
                            start=True,
                            stop=False,
                        )
                if bf16_split:
                    # x @ W = (xh+xl)(Wh+Wl); the dropped xl.Wl term is
                    # ~1e-4 of a logit — verified not to flip any argmax for
                    # this model's weight/input scale. Both groups stream
                    # under each loaded x chunk.
                    steps = [
                        (xh_t, wh_t, ki) for ki in range(KCH)
                    ] + [(xh_t, wl_t, ki) for ki in range(KCH)] + [
                        (xl_t, wh_t, ki) for ki in range(KCH)
                    ]
                else:
                    steps = [(x_t, wt_t, ki) for ki in range(KCH)]
                for si, (xa, wa, k) in enumerate(steps):
                    for g in range(GROUPS):
                        nc.tensor.matmul(
                            out=lg[g][:ssz, :],
                            lhsT=xa[:, k, s0 : s0 + ssz],
                            rhs=wa[:, k, g * NUM_VARS : (g + 1) * NUM_VARS],
                            start=(si == 0 and not has_bias),
                            stop=(si == len(steps) - 1),
                        )

                mx = small_p.tile([128, GROUPS], F32, tag="mx")
                negmx = small_p.tile([128, GROUPS], F32, tag="negmx")
                se = small_p.tile([128, GROUPS], F32, tag="se")
                rse = rse_p.tile([128, GROUPS], BF16, tag="rse")
                exps = sm_p.tile([128, GV], BF16, tag="exps")
                exps_list.append(exps)
                rse_list.append(rse)

                for g in range(GROUPS):
                    nc.vector.tensor_reduce(
                        out=mx[:ssz, g : g + 1],
                        in_=lg[g][:ssz, :],
                        axis=mybir.AxisListType.X,
                        op=OP.max,
                    )
                if do_softmax:
                    nc.vector.tensor_scalar_mul(negmx[:ssz, :], mx[:ssz, :], -1.0)
                for g in range(GROUPS):
                    if do_softmax:
                        nc.scalar.activation(
                            out=exps[:ssz, g * NUM_VARS : (g + 1) * NUM_VARS],
                            in_=lg[g][:ssz, :],
                            func=AF.Exp,
                            bias=negmx[:ssz, g : g + 1],
                            scale=1.0,
                            accum_out=se[:ssz, g : g + 1],
                        )
                    if do_argmax:
                        # idx = sum_v (lg_v == mx) * iota_v in a single DVE op;
                        # iota's base already folds in the group's codebook row
                        # offset.
                        scr = oh_p.tile([128, NUM_VARS], F32, tag="scr")
                        nc.vector.scalar_tensor_tensor(
                            out=scr[:ssz, :],
                            in0=lg[g][:ssz, :],
                            scalar=mx[:ssz, g : g + 1],
                            in1=iota_ts[g][:ssz, :],
                            op0=OP.is_equal,
                            op1=OP.mult,
                            accum_out=idxf[:ssz, s * GROUPS + g : s * GROUPS + g + 1],
                        )

                if do_softmax:
                    with nc.allow_low_precision(
                        reason="1/sumexp in bf16 feeds an averaged matmul"
                    ):
                        nc.vector.reciprocal(rse[:ssz, :], se[:ssz, :])

            if do_avg and do_softmax:
                # Batched after the subtile loop so the PE never stalls
                # waiting on the softmax chain mid-stream; inputs are bf16
                # (plenty for an average of probabilities) so these matmuls
                # run at full rate.
                for s in range(nsub):
                    ssz = min(128, tsz - s * 128)
                    last = t == N_OUTER - 1 and s == nsub - 1
                    for g in range(GROUPS):
                        nc.tensor.matmul(
                            out=avg_ps[g][:GROUPS, :],
                            lhsT=rse_list[s][:ssz, :],
                            rhs=exps_list[s][:ssz, g * NUM_VARS : (g + 1) * NUM_VARS],
                            start=(t == 0 and s == 0),
                            stop=last,
                        )

            if do_argmax:
                nc.vector.tensor_copy(out=idxi[:, : 2 * nsub], in_=idxf[:, : 2 * nsub])
            else:
                nc.gpsimd.memset(idxi[:], 0)

            if do_gather:
                # multi-index-per-partition offsets crash the device; one
                # gather per (subtile, group) column.
                for j in range(2 * nsub):
                    nc.gpsimd.indirect_dma_start(
                        out=q_t[:, j, :],
                        out_offset=None,
                        in_=cb[:, :],
                        in_offset=bass.IndirectOffsetOnAxis(
                            ap=idxi[:, j : j + 1], axis=0
                        ),
                    )
            else:
                nc.gpsimd.memset(q_t[:], 0.0)

            for s in range(nsub):
                s0 = s * 128
                ssz = min(128, tsz - s0)
                nc.sync.dma_start(
                    out=q_out[tok0 + s0 : tok0 + s0 + ssz, :].rearrange(
                        "t (g d) -> t g d", d=VAR_DIM
                    ),
                    in_=q_t[:ssz, s * GROUPS : (s + 1) * GROUPS, :],
                )

        # DVE reads must start at partition 0, so copy both PSUM rows per
        # group and let the output DMA select row g.
        avg_sb = const_p.tile([GROUPS, GROUPS * NUM_VARS], F32, tag="avg_sb")
        if not (do_avg and do_softmax):
            nc.gpsimd.memset(avg_sb[:], 0.0)
        for g in range(GROUPS):
            if do_avg and do_softmax:
                nc.vector.tensor_copy(
                    out=avg_sb[:GROUPS, g * NUM_VARS : (g + 1) * NUM_VARS],
                    in_=avg_ps[g][:GROUPS, :],
                )
            nc.sync.dma_start(
                out=avgp[g : g + 1, :],
                in_=avg_sb[g : g + 1, g * NUM_VARS : (g + 1) * NUM_VARS],
            )

    nc.compile()
    return nc


_PROGRAM_CACHE: dict[bool, bass.Bass] = {}


BF16_SPLIT = True


def _prepare(x, W, b, codebook):
    import ml_dtypes

    x = np.ascontiguousarray(np.asarray(x, dtype=np.float32))
    W = np.asarray(W, dtype=np.float32)
    b = np.asarray(b, dtype=np.float32)
    cbk = np.asarray(codebook, dtype=np.float32).reshape(GV, VAR_DIM)
    has_bias = bool(np.any(b != 0.0))

    bf = ml_dtypes.bfloat16
    cbc = np.ascontiguousarray(cbk)  # [GV, VAR_DIM]
    per_b = B // N_CORES
    in_maps = []
    if BF16_SPLIT:
        WT = W.T.astype(np.float32)  # [D, GV]
        WTh32 = WT.astype(bf).astype(np.float32)
        WTh = np.ascontiguousarray(WTh32.astype(bf))
        WTl = np.ascontiguousarray((WT - WTh32).astype(bf))
    else:
        WTc = np.ascontiguousarray(W.T)
    for i in range(N_CORES):
        xs = x[i * per_b : (i + 1) * per_b].reshape(NTOK, D)
        xT = np.ascontiguousarray(xs.T)
        if BF16_SPLIT:
            xh32 = xT.astype(bf).astype(np.float32)
            m = {
                "xTh": np.ascontiguousarray(xh32.astype(bf)),
                "xTl": np.ascontiguousarray((xT - xh32).astype(bf)),
                "WTh": WTh,
                "WTl": WTl,
                "cb": cbc,
            }
        else:
            m = {"xT": xT, "WT": WTc, "cb": cbc}
        if has_bias:
            m["bv"] = np.ascontiguousarray(b.reshape(1, GV))
        in_maps.append(m)
    return has_bias, in_maps


def _postprocess(results):
    q = np.concatenate([results[i]["q_out"] for i in range(N_CORES)], axis=0)
    q = q.reshape(B, T, VQ_DIM)
    avg = np.zeros((GROUPS, NUM_VARS), dtype=np.float64)
    for i in range(N_CORES):
        avg += results[i]["avgp"].astype(np.float64)
    avg_probs = (avg / (B * T)).astype(np.float32)
    return q, avg_probs


def _get_program(has_bias: bool) -> bass.Bass:
    nc = _PROGRAM_CACHE.get(has_bias)
    if nc is None:
        nc = _build_program(has_bias, bf16_split=BF16_SPLIT)
        _PROGRAM_CACHE[has_bias] = nc
    return nc


def kernel(x, W, b, codebook):
    has_bias, in_maps = _prepare(x, W, b, codebook)
    nc = _get_program(has_bias)
    results = run_bass_kernel_spmd(nc, in_maps, list(range(N_CORES))).results
    return _postprocess(results)
